# revision 60
# baseline (speedup 1.0000x reference)
"""Trainium2 Bass kernel for nn_MultiHeadLocalAttention (band-limited attention).

Math: scores are multiplied by a band-count matrix C that is zero outside
|q-k|<=4, then a FULL-row softmax is applied.  Out-of-band entries contribute
exp(0)=1, so with E = exp(C*S) over the 136-wide halo band:

  out[q]   = (sum_halo (E[k,q]-1) vh[k] + VsumW) / denom[q]
  denom[q] = sum_halo (E[k,q]-1) + 2048

where VsumW = (sum_k v[k]) @ Wv (computed host-side in fp64).  This makes the
O(seq^2) attention an O(seq*band) computation, exact to fp rounding.

Sharding: batch*seq rows split across 8 cores (512 rows each); each core
computes all 8 heads for its rows.  k/v inputs carry a +-4 halo.  All matmul
operands are bf16 (1 PE cycle/row vs 4 for fp32); PSUM accumulation is fp32.
"""

import math
import sys
from contextlib import ExitStack

import numpy as np

sys.path.insert(0, "/opt/trn_rl_repo")

import ml_dtypes

import concourse.bass as bass
import concourse.tile as tile
from concourse import bacc, mybir
from concourse.bass_utils import run_bass_kernel_spmd

F32 = mybir.dt.float32
BF16 = mybir.dt.bfloat16
NP_BF16 = ml_dtypes.bfloat16
SEQ, DM, H, DK = 2048, 512, 8, 64
ROWS = 512          # query rows per core
HALO = ROWS + 8     # padded k/v halo columns per core
J = 4               # 128-row query tiles per core
WBAND = 2


# ----------------------------------------------------------------------------
# host-side helpers
# ----------------------------------------------------------------------------

def _band_count(seq=SEQ, window=WBAND):
    i = np.arange(seq)
    lo = np.clip(i - window, 0, None)
    hi = np.clip(i + window, None, seq - 1)
    lo = np.where(i == 1, 0, lo)
    hi = np.where(i == 1, window + 1, hi)
    lo = np.where(i == seq - 2, seq - window - 2, lo)
    hi = np.where(i == seq - 2, seq - 1, hi)
    a = np.arange(seq)[None, :]
    M = ((a >= lo[:, None]) & (a <= hi[:, None])).astype(np.float32)
    return M.T @ M


def _c_tiles(R0, C):
    """CA [J,128,128], CB [J,8,128] with C[k,q]/sqrt(dk) (C symmetric)."""
    CA = np.zeros((J, 128, 128), np.float32)
    CB = np.zeros((J, 8, 128), np.float32)
    for j in range(J):
        qg = R0 + 128 * j + np.arange(128)
        kgA = R0 - 4 + 128 * j + np.arange(128)
        kgB = R0 - 4 + 128 * j + 128 + np.arange(8)
        mA = (kgA >= 0) & (kgA < SEQ)
        mB = (kgB >= 0) & (kgB < SEQ)
        CA[j][mA, :] = C[np.ix_(kgA[mA], qg)]
        CB[j][mB, :] = C[np.ix_(kgB[mB], qg)]
    return CA, CB


# ----------------------------------------------------------------------------
# device program
# ----------------------------------------------------------------------------

def _build_program(with_bias, upto=4):
    nc = bacc.Bacc()
    E_IN = dict(kind="ExternalInput")
    qT_d = nc.dram_tensor("qT", [DM, ROWS], BF16, **E_IN)
    kT_d = nc.dram_tensor("kT", [DM, HALO], BF16, **E_IN)
    vT_d = nc.dram_tensor("vT", [DM, HALO], BF16, **E_IN)
    Wq_d = nc.dram_tensor("Wq", [DM, DM], BF16, **E_IN)
    Wk_d = nc.dram_tensor("Wk", [DM, DM], BF16, **E_IN)
    Wv_d = nc.dram_tensor("Wv", [DM, DM], BF16, **E_IN)
    Wo_d = nc.dram_tensor("Wo", [DM, DM], BF16, **E_IN)
    CA_d = nc.dram_tensor("CA", [J, 128, 128], BF16, **E_IN)
    CBs_d = nc.dram_tensor("CBs", [J, 8, 8], BF16, **E_IN)
    vsW_d = nc.dram_tensor("vsW65", [1, H * 65], BF16, **E_IN)
    ident_d = nc.dram_tensor("ident", [128, 128], BF16, **E_IN)
    if with_bias:
        bq_d = nc.dram_tensor("bq", [1, DM], BF16, **E_IN)
        bk_d = nc.dram_tensor("bk", [1, DM], BF16, **E_IN)
        bv_d = nc.dram_tensor("bv", [1, DM], BF16, **E_IN)
    out_d = nc.dram_tensor("out", [ROWS, DM], BF16, kind="ExternalOutput")

    MULT = mybir.AluOpType.mult
    EXP = mybir.ActivationFunctionType.Exp

    with tile.TileContext(nc) as tc, ExitStack() as ctx:
        sing = ctx.enter_context(tc.tile_pool(name="sing", bufs=1))

        # --- load everything to SBUF (q-proj operands first), split across
        # the sync and scalar HWDGE queues; small tables ride gpsimd SWDGE ---
        def dual(sb, ap):
            # halves across both HWDGE queues -> tensor ready in half the time
            nc.sync.dma_start(sb[0:64], ap[0:64])
            nc.scalar.dma_start(sb[64:128], ap[64:128])

        sb_Wq = sing.tile([128, 4, DM], BF16)
        dual(sb_Wq, Wq_d[:].rearrange("(kc p) n -> p kc n", p=128))
        sb_qT = sing.tile([128, 4, ROWS], BF16)
        dual(sb_qT, qT_d[:].rearrange("(kc p) r -> p kc r", p=128))
        sb_kT = sing.tile([128, 4, HALO], BF16)
        dual(sb_kT, kT_d[:].rearrange("(kc p) r -> p kc r", p=128))
        sb_Wk = sing.tile([128, 4, DM], BF16)
        dual(sb_Wk, Wk_d[:].rearrange("(kc p) n -> p kc n", p=128))
        sb_Wv = sing.tile([128, 4, DM], BF16)
        nc.sync.dma_start(sb_Wv, Wv_d[:].rearrange("(kc p) n -> p kc n", p=128))
        sb_vT = sing.tile([128, 4, HALO], BF16)
        nc.scalar.dma_start(sb_vT, vT_d[:].rearrange("(kc p) r -> p kc r", p=128))
        sb_CA = sing.tile([128, J, 128], BF16)
        nc.sync.dma_start(sb_CA, CA_d[:].rearrange("j p q -> p j q"))
        sb_Wo = sing.tile([128, 4, DM], BF16)
        nc.scalar.dma_start(sb_Wo, Wo_d[:].rearrange("(kc p) n -> p kc n", p=128))
        sb_CBs = sing.tile([8, J, 8], BF16)
        nc.gpsimd.dma_start(sb_CBs, CBs_d[:].rearrange("j p q -> p j q"))
        sb_vs65 = sing.tile([1, H, 65], BF16)     # [VsumW_h, 2048] per head
        nc.gpsimd.dma_start(sb_vs65, vsW_d[:].rearrange("p (h d) -> p h d", h=H))
        sb_id = sing.tile([128, 128], BF16)
        nc.gpsimd.dma_start(sb_id, ident_d[:])
        if with_bias:
            sb_bq = sing.tile([1, DM], BF16)
            nc.gpsimd.dma_start(sb_bq, bq_d[:])
            sb_bk = sing.tile([1, DM], BF16)
            nc.gpsimd.dma_start(sb_bk, bk_d[:])
            sb_bv = sing.tile([1, DM], BF16)
            nc.gpsimd.dma_start(sb_bv, bv_d[:])

        sb_ones_r = sing.tile([1, ROWS], BF16)    # ones row (rhs for bias/Vsum)
        nc.vector.memset(sb_ones_r, 1.0)
        sb_eB = sing.tile([8, H * 128], BF16)     # exp(C*S)-1 piece B; 0 off band
        nc.vector.memset(sb_eB, 0.0)

        # persistent intermediates (qhT/khT head-separated: all matmul
        # operands live at partition base 0 -- avoids PE row-group switching)
        sb_qhT = sing.tile([64, H, ROWS], BF16)
        sb_khT = sing.tile([64, H, HALO], BF16)
        sb_vh = sing.tile([128, 5, H, 65], BF16)  # shifted row tiles, 65 = 64d+1
        sb_concat = sing.tile([128, J, DM], BF16)
        sb_concatT = sing.tile([128, 4, ROWS], BF16)

        # ---------------- phase 1: projections ----------------
        with tc.tile_pool(name="ppj", bufs=2, space="PSUM") as ppj, \
             tc.tile_pool(name="ppv", bufs=2, space="PSUM") as ppv, \
             tc.tile_pool(name="ppt", bufs=1, space="PSUM") as ppt:
            # qhT[h][64 dout, rows]: head pairs per 2-bank psum tile
            for hp2 in range(4):
                ps = ppj.tile([64, 2, ROWS], F32, tag="pj")
                for i in range(2):
                    hh = 2 * hp2 + i
                    for kc in range(4):
                        nc.tensor.matmul(ps[:, i, :],
                                         sb_Wq[:, kc, 64 * hh:64 * hh + 64],
                                         sb_qT[:, kc, :], start=(kc == 0),
                                         stop=(kc == 3 and not with_bias))
                    if with_bias:
                        nc.tensor.matmul(ps[:, i, :],
                                         sb_bq[0:1, 64 * hh:64 * hh + 64],
                                         sb_ones_r, start=False, stop=True)
                nc.scalar.copy(sb_qhT[:, 2 * hp2:2 * hp2 + 2, :], ps)
            # khT[h][64 dout, halo]: main 512 cols paired; 8-col tails separate
            for hp2 in range(4):
                ps = ppj.tile([64, 2, ROWS], F32, tag="pj")
                pst = ppt.tile([64, 2, 8], F32, tag="pt")
                for i in range(2):
                    hh = 2 * hp2 + i
                    for kc in range(4):
                        nc.tensor.matmul(ps[:, i, :],
                                         sb_Wk[:, kc, 64 * hh:64 * hh + 64],
                                         sb_kT[:, kc, 0:512], start=(kc == 0),
                                         stop=(kc == 3 and not with_bias))
                        nc.tensor.matmul(pst[:, i, :],
                                         sb_Wk[:, kc, 64 * hh:64 * hh + 64],
                                         sb_kT[:, kc, 512:HALO], start=(kc == 0),
                                         stop=(kc == 3 and not with_bias))
                    if with_bias:
                        nc.tensor.matmul(ps[:, i, :],
                                         sb_bk[0:1, 64 * hh:64 * hh + 64],
                                         sb_ones_r, start=False, stop=True)
                        nc.tensor.matmul(pst[:, i, :],
                                         sb_bk[0:1, 64 * hh:64 * hh + 64],
                                         sb_ones_r[0:1, 0:8], start=False, stop=True)
                nc.vector.tensor_copy(sb_khT[:, 2 * hp2:2 * hp2 + 2, 0:512], ps)
                nc.vector.tensor_copy(sb_khT[:, 2 * hp2:2 * hp2 + 2, 512:HALO], pst)
            # vh[rows(+halo shift), dout] in 65-strided head blocks
            for rt in range(5):
                nr = 128 if rt < 4 else 8
                ps = ppv.tile([128, DM], F32, tag="pv")
                for kc in range(4):
                    nc.tensor.matmul(ps[0:nr, :],
                                     sb_vT[:, kc, 128 * rt:128 * rt + nr],
                                     sb_Wv[:, kc, :], start=(kc == 0),
                                     stop=(kc == 3 and not with_bias))
                if with_bias:
                    nc.tensor.matmul(ps[0:nr, :], sb_ones_r[0:1, 0:nr], sb_bv,
                                     start=False, stop=True)
                eng = nc.scalar.copy if rt % 2 == 0 else nc.vector.tensor_copy
                eng(sb_vh[0:nr, rt, :, 0:64],
                    ps[0:nr, :].rearrange("p (h d) -> p h d", h=H))
            nc.vector.memset(sb_vh[:, :, :, 64:65], 1.0)

        # ---------------- phase 2: band attention ----------------
        with tc.tile_pool(name="pSA", bufs=2, space="PSUM") as pSA, \
             tc.tile_pool(name="pSB", bufs=1, space="PSUM") as pSB, \
             tc.tile_pool(name="pN", bufs=3, space="PSUM") as pN, \
             tc.tile_pool(name="att", bufs=2) as att:
            for j in range(J if upto >= 2 else 0):
                psa = pSA.tile([128, H * 128], F32, tag="sa")
                psb = pSB.tile([8, H * 8], F32, tag="sb")
                for h in range(H):
                    qh_j = sb_qhT[:, h, 128 * j:128 * j + 128]
                    nc.tensor.matmul(psa[:, 128 * h:128 * h + 128],
                                     sb_khT[:, h, 128 * j:128 * j + 128],
                                     qh_j, start=True, stop=True)
                    # band of piece B only touches query cols 120:128
                    nc.tensor.matmul(psb[:, 8 * h:8 * h + 8],
                                     sb_khT[:, h, 128 * j + 128:128 * j + 136],
                                     sb_qhT[:, h, 128 * j + 120:128 * j + 128],
                                     start=True, stop=True)
                # CS = C * S (in place), E = exp(CS) - 1 -> SBUF (bf16)
                ca = sb_CA[:, j, :]
                ca_b = bass.AP(tensor=ca.tensor, offset=ca.offset,
                               ap=[list(ca.ap[0]), [0, H], list(ca.ap[1])])
                psa_v = psa[:].rearrange("p (h q) -> p h q", h=H)
                nc.vector.tensor_mul(psa_v, psa_v, ca_b)
                cb = sb_CBs[:, j, :]
                cb_b = bass.AP(tensor=cb.tensor, offset=cb.offset,
                               ap=[list(cb.ap[0]), [0, H], list(cb.ap[1])])
                psb_v = psb[:].rearrange("p (h q) -> p h q", h=H)
                nc.vector.tensor_mul(psb_v, psb_v, cb_b)
                eA = att.tile([128, H * 128], BF16, tag="eA")
                nc.scalar.activation(eA, psa, EXP)
                nc.vector.tensor_scalar_add(eA, eA, -1.0)
                eB_v = sb_eB[:].rearrange("p (h q) -> p h q", h=H)[:, :, 120:128]
                nc.scalar.activation(eB_v, psb_v, EXP)
                nc.scalar.activation(eB_v, eB_v,
                                     mybir.ActivationFunctionType.Copy, bias=-1.0)
                # numer: [128, 65] per head, 4 heads per psum tile
                pn0 = pN.tile([128, 4, 65], F32, tag="n")
                pn1 = pN.tile([128, 4, 65], F32, tag="n")
                for h in range(H):
                    pn = (pn0 if h < 4 else pn1)[:, h % 4, :]
                    nc.tensor.matmul(pn, eA[:, 128 * h:128 * h + 128],
                                     sb_vh[:, j, h, :], start=True, stop=False)
                    nc.tensor.matmul(pn, sb_eB[:, 128 * h:128 * h + 128],
                                     sb_vh[0:8, j + 1, h, :], start=False,
                                     stop=False)
                    nc.tensor.matmul(pn, sb_ones_r[0:1, 0:128],
                                     sb_vs65[0:1, h, :], start=False, stop=True)
                # divide + write concat
                r = att.tile([128, H], F32, tag="r")
                nc.vector.reciprocal(r[:, 0:4], pn0[:, :, 64])
                nc.vector.reciprocal(r[:, 4:8], pn1[:, :, 64])
                for pn, ho in ((pn0, 0), (pn1, 4)):
                    rs = r[:, ho:ho + 4]
                    r_b = bass.AP(tensor=rs.tensor, offset=rs.offset,
                                  ap=[list(rs.ap[0]), list(rs.ap[1]), [0, 64]])
                    outv = sb_concat[:, j, 64 * ho:64 * ho + 256]
                    nc.vector.scalar_tensor_tensor(
                        outv.rearrange("p (h d) -> p h d", h=4),
                        pn[:, :, 0:64], 1.0, r_b, op0=MULT, op1=MULT)

        # ---------------- phase 3: transpose concat ----------------
        with tc.tile_pool(name="pT", bufs=2, space="PSUM") as pT:
            for dc in range(4 if upto >= 3 else 0):
                for rc in range(4):
                    pt = pT.tile([128, 128], BF16, tag="t")
                    nc.tensor.transpose(pt, sb_concat[:, rc, 128 * dc:128 * dc + 128],
                                        sb_id)
                    eng = nc.scalar.copy if (dc + rc) % 2 == 0 else nc.vector.tensor_copy
                    eng(sb_concatT[:, dc, 128 * rc:128 * rc + 128], pt)

        # ---------------- phase 4: output projection ----------------
        with tc.tile_pool(name="pF", bufs=2, space="PSUM") as pF, \
             tc.tile_pool(name="fout", bufs=2) as fout:
            if upto < 4:
                src_t = {2: sb_concat, 3: sb_concatT}.get(upto, sb_concatT)
                for rc in range(4):
                    if upto == 1:
                        nc.sync.dma_start(out_d[128 * rc:128 * rc + 128, 0:256],
                                          sb_vh[:, rc, 0:4, 0:64])
                    else:
                        nc.sync.dma_start(out_d[128 * rc:128 * rc + 128, :],
                                          src_t[:, rc, 0:DM])
            for rc in range(4 if upto >= 4 else 0):
                pf = pF.tile([128, DM], F32, tag="f")
                for dc in range(4):
                    nc.tensor.matmul(pf, sb_concatT[:, dc, 128 * rc:128 * rc + 128],
                                     sb_Wo[:, dc, :], start=(dc == 0), stop=(dc == 3))
                so = fout.tile([128, DM], BF16, tag="fo")
                nc.scalar.copy(so, pf)
                dq = nc.sync if rc % 2 == 0 else nc.scalar
                dq.dma_start(out_d[128 * rc:128 * rc + 128, :], so)

    if not nc.is_finalized():
        nc.finalize()
    return nc


_PROG_CACHE = {}


def _get_program(with_bias):
    import os
    upto = int(os.environ.get("BASS_KERNEL_UPTO", "4"))
    key = (bool(with_bias), upto)
    if key not in _PROG_CACHE:
        _PROG_CACHE[key] = _build_program(with_bias, upto)
    return _PROG_CACHE[key]


# ----------------------------------------------------------------------------
# entry point
# ----------------------------------------------------------------------------

def prep_in_maps(q, k, v, Wq, bq, Wk, bk, Wv, bv, Wo, bo, **_unused):
    """Builds per-core input maps + the traced program; returns (in_maps, nc)."""
    q = np.asarray(q, np.float32)
    k = np.asarray(k, np.float32)
    v = np.asarray(v, np.float32)
    Wq_b = np.ascontiguousarray(Wq, np.float32).astype(NP_BF16)
    Wk_b = np.ascontiguousarray(Wk, np.float32).astype(NP_BF16)
    Wv_b = np.ascontiguousarray(Wv, np.float32).astype(NP_BF16)
    Wo_b = np.ascontiguousarray(Wo, np.float32).astype(NP_BF16)
    bq = np.asarray(bq, np.float32).reshape(-1)
    bk = np.asarray(bk, np.float32).reshape(-1)
    bv = np.asarray(bv, np.float32).reshape(-1)
    bo = np.asarray(bo, np.float32).reshape(-1)
    with_bias = bool(np.any(bq) or np.any(bk) or np.any(bv))
    nc = _get_program(with_bias)

    C = _band_count() / np.float32(math.sqrt(DK))
    ident = np.eye(128, dtype=NP_BF16)
    # VsumW = (sum_k v[k]) @ Wv (+ seq*bv) in fp64, per batch
    vsum = v.sum(axis=1, dtype=np.float64)  # [2, 512]
    vsW = vsum @ Wv_b.astype(np.float64) + float(SEQ) * bv.astype(np.float64)

    in_maps = []
    for c in range(8):
        b, R0 = c // 4, ROWS * (c % 4)
        qT = np.ascontiguousarray(q[b, R0:R0 + ROWS, :].T).astype(NP_BF16)
        kT = np.zeros((DM, HALO), NP_BF16)
        vT = np.zeros((DM, HALO), NP_BF16)
        g0 = R0 - 4
        s0, s1 = max(g0, 0), min(R0 + ROWS + 4, SEQ)
        kT[:, s0 - g0:s1 - g0] = k[b, s0:s1, :].T.astype(NP_BF16)
        vT[:, s0 - g0:s1 - g0] = v[b, s0:s1, :].T.astype(NP_BF16)
        CA, CB = _c_tiles(R0, C)
        CBs = np.ascontiguousarray(CB[:, :, 120:128]).astype(NP_BF16)
        assert not CB[:, :, :120].any()
        vs65 = np.empty((H, 65), np.float64)
        vs65[:, 0:64] = vsW[b].reshape(H, 64)
        vs65[:, 64] = float(SEQ)
        m = {"qT": qT, "kT": kT, "vT": vT, "Wq": Wq_b, "Wk": Wk_b, "Wv": Wv_b,
             "Wo": Wo_b, "CA": CA.astype(NP_BF16), "CBs": CBs,
             "vsW65": vs65.reshape(1, H * 65).astype(NP_BF16),
             "ident": ident}
        if with_bias:
            m["bq"] = bq[None, :].astype(NP_BF16)
            m["bk"] = bk[None, :].astype(NP_BF16)
            m["bv"] = bv[None, :].astype(NP_BF16)
        in_maps.append(m)
    return in_maps, nc


def kernel(q, k, v, Wq, bq, Wk, bk, Wv, bv, Wo, bo, **_unused):
    bo = np.asarray(bo, np.float32).reshape(-1)
    in_maps, nc = prep_in_maps(q, k, v, Wq, bq, Wk, bk, Wv, bv, Wo, bo)
    res = run_bass_kernel_spmd(nc, in_maps, core_ids=list(range(8)))
    out = np.empty((2, SEQ, DM), np.float32)
    for c in range(8):
        b, R0 = c // 4, ROWS * (c % 4)
        out[b, R0:R0 + ROWS, :] = res.results[c]["out"].astype(np.float32)
    if np.any(bo):
        out += bo
    return out


if __name__ == "__main__":
    rng = np.random.default_rng(0)
    s = 1.0 / math.sqrt(DM)
    inp = dict(
        q=rng.standard_normal((2, SEQ, DM)).astype(np.float32),
        k=rng.standard_normal((2, SEQ, DM)).astype(np.float32),
        v=rng.standard_normal((2, SEQ, DM)).astype(np.float32),
        Wq=(rng.standard_normal((DM, DM)) * s).astype(np.float32),
        bq=np.zeros(DM, np.float32),
        Wk=(rng.standard_normal((DM, DM)) * s).astype(np.float32),
        bk=np.zeros(DM, np.float32),
        Wv=(rng.standard_normal((DM, DM)) * s).astype(np.float32),
        bv=np.zeros(DM, np.float32),
        Wo=(rng.standard_normal((DM, DM)) * s).astype(np.float32),
        bo=np.zeros(DM, np.float32),
    )
    out = kernel(**inp)
    print("kernel ran, out shape", out.shape, "mean", np.abs(out).mean())


# revision 66
# speedup vs baseline: 1.0207x; 1.0207x over previous
"""Trainium2 Bass kernel for nn_MultiHeadLocalAttention (band-limited attention).

Math: scores are multiplied by a band-count matrix C that is zero outside
|q-k|<=4, then a FULL-row softmax is applied.  Out-of-band entries contribute
exp(0)=1, so with E = exp(C*S) over the 136-wide halo band:

  out[q]   = (sum_halo (E[k,q]-1) vh[k] + VsumW) / denom[q]
  denom[q] = sum_halo (E[k,q]-1) + 2048

where VsumW = (sum_k v[k]) @ Wv (computed host-side in fp64).  This makes the
O(seq^2) attention an O(seq*band) computation, exact to fp rounding.

Sharding: batch*seq rows split across 8 cores (512 rows each); each core
computes all 8 heads for its rows.  k/v inputs carry a +-4 halo.  All matmul
operands are bf16 (1 PE cycle/row vs 4 for fp32); PSUM accumulation is fp32.
"""

import math
import sys
from contextlib import ExitStack

import numpy as np

sys.path.insert(0, "/opt/trn_rl_repo")

import ml_dtypes

import concourse.bass as bass
import concourse.tile as tile
from concourse import bacc, mybir
from concourse.bass_utils import run_bass_kernel_spmd

F32 = mybir.dt.float32
BF16 = mybir.dt.bfloat16
NP_BF16 = ml_dtypes.bfloat16
SEQ, DM, H, DK = 2048, 512, 8, 64
ROWS = 512          # query rows per core
HALO = ROWS + 8     # padded k/v halo columns per core
J = 4               # 128-row query tiles per core
WBAND = 2


# ----------------------------------------------------------------------------
# host-side helpers
# ----------------------------------------------------------------------------

def _band_count(seq=SEQ, window=WBAND):
    i = np.arange(seq)
    lo = np.clip(i - window, 0, None)
    hi = np.clip(i + window, None, seq - 1)
    lo = np.where(i == 1, 0, lo)
    hi = np.where(i == 1, window + 1, hi)
    lo = np.where(i == seq - 2, seq - window - 2, lo)
    hi = np.where(i == seq - 2, seq - 1, hi)
    a = np.arange(seq)[None, :]
    M = ((a >= lo[:, None]) & (a <= hi[:, None])).astype(np.float32)
    return M.T @ M


def _c_tiles(R0, C):
    """CA [J,128,128], CB [J,8,128] with C[k,q]/sqrt(dk) (C symmetric)."""
    CA = np.zeros((J, 128, 128), np.float32)
    CB = np.zeros((J, 8, 128), np.float32)
    for j in range(J):
        qg = R0 + 128 * j + np.arange(128)
        kgA = R0 - 4 + 128 * j + np.arange(128)
        kgB = R0 - 4 + 128 * j + 128 + np.arange(8)
        mA = (kgA >= 0) & (kgA < SEQ)
        mB = (kgB >= 0) & (kgB < SEQ)
        CA[j][mA, :] = C[np.ix_(kgA[mA], qg)]
        CB[j][mB, :] = C[np.ix_(kgB[mB], qg)]
    return CA, CB


# ----------------------------------------------------------------------------
# device program
# ----------------------------------------------------------------------------

def _build_program(with_bias, upto=4):
    nc = bacc.Bacc()
    E_IN = dict(kind="ExternalInput")
    qT_d = nc.dram_tensor("qT", [DM, ROWS], BF16, **E_IN)
    kT_d = nc.dram_tensor("kT", [DM, HALO], BF16, **E_IN)
    vT_d = nc.dram_tensor("vT", [DM, HALO], BF16, **E_IN)
    Wq_d = nc.dram_tensor("Wq", [DM, DM], BF16, **E_IN)
    Wk_d = nc.dram_tensor("Wk", [DM, DM], BF16, **E_IN)
    Wv_d = nc.dram_tensor("Wv", [DM, DM], BF16, **E_IN)
    Wo_d = nc.dram_tensor("Wo", [DM, DM], BF16, **E_IN)
    CA_d = nc.dram_tensor("CA", [J, 128, 128], BF16, **E_IN)
    CBs_d = nc.dram_tensor("CBs", [J, 8, 8], BF16, **E_IN)
    vsW_d = nc.dram_tensor("vsW65", [1, H * 65], BF16, **E_IN)
    ident_d = nc.dram_tensor("ident", [128, 128], BF16, **E_IN)
    if with_bias:
        bq_d = nc.dram_tensor("bq", [1, DM], BF16, **E_IN)
        bk_d = nc.dram_tensor("bk", [1, DM], BF16, **E_IN)
        bv_d = nc.dram_tensor("bv", [1, DM], BF16, **E_IN)
    out_d = nc.dram_tensor("out", [ROWS, DM], BF16, kind="ExternalOutput")

    MULT = mybir.AluOpType.mult
    EXP = mybir.ActivationFunctionType.Exp

    with tile.TileContext(nc) as tc, ExitStack() as ctx:
        sing = ctx.enter_context(tc.tile_pool(name="sing", bufs=1))

        # --- load everything to SBUF (q-proj operands first), split across
        # the sync and scalar HWDGE queues; small tables ride gpsimd SWDGE ---
        def dual_kc(sb, dram, n):
            # kc-block halves: both DRAM slices stay fully contiguous
            a = dram[:].rearrange("(kc p) n -> p kc n", p=128)
            nc.sync.dma_start(sb[:, 0:2, :], dram[0:256].rearrange(
                "(kc p) n -> p kc n", p=128))
            nc.scalar.dma_start(sb[:, 2:4, :], dram[256:512].rearrange(
                "(kc p) n -> p kc n", p=128))

        sb_Wq = sing.tile([128, 4, DM], BF16)
        dual_kc(sb_Wq, Wq_d, DM)
        sb_qT = sing.tile([128, 4, ROWS], BF16)
        dual_kc(sb_qT, qT_d, ROWS)
        sb_kT = sing.tile([128, 4, HALO], BF16)
        dual_kc(sb_kT, kT_d, HALO)
        sb_Wk = sing.tile([128, 4, DM], BF16)
        dual_kc(sb_Wk, Wk_d, DM)
        sb_Wv = sing.tile([128, 4, DM], BF16)
        nc.sync.dma_start(sb_Wv, Wv_d[:].rearrange("(kc p) n -> p kc n", p=128))
        sb_vT = sing.tile([128, 4, HALO], BF16)
        nc.scalar.dma_start(sb_vT, vT_d[:].rearrange("(kc p) r -> p kc r", p=128))
        sb_CA = sing.tile([128, J, 128], BF16)
        nc.sync.dma_start(sb_CA, CA_d[:].rearrange("j p q -> p j q"))
        sb_Wo = sing.tile([128, 4, DM], BF16)
        nc.scalar.dma_start(sb_Wo, Wo_d[:].rearrange("(kc p) n -> p kc n", p=128))
        sb_CBs = sing.tile([8, J, 8], BF16)
        nc.gpsimd.dma_start(sb_CBs, CBs_d[:].rearrange("j p q -> p j q"))
        sb_vs65 = sing.tile([1, H, 65], BF16)     # [VsumW_h, 2048] per head
        nc.gpsimd.dma_start(sb_vs65, vsW_d[:].rearrange("p (h d) -> p h d", h=H))
        sb_id = sing.tile([128, 128], BF16)
        nc.gpsimd.dma_start(sb_id, ident_d[:])
        if with_bias:
            sb_bq = sing.tile([1, DM], BF16)
            nc.gpsimd.dma_start(sb_bq, bq_d[:])
            sb_bk = sing.tile([1, DM], BF16)
            nc.gpsimd.dma_start(sb_bk, bk_d[:])
            sb_bv = sing.tile([1, DM], BF16)
            nc.gpsimd.dma_start(sb_bv, bv_d[:])

        sb_ones_r = sing.tile([1, ROWS], BF16)    # ones row (rhs for bias/Vsum)
        nc.vector.memset(sb_ones_r, 1.0)
        sb_warm = sing.tile([128, 512], BF16)     # HAM-filler matmul scratch
        nc.vector.memset(sb_warm, 0.0)
        sb_eB = sing.tile([8, H * 128], BF16)     # exp(C*S)-1 piece B; 0 off band
        nc.vector.memset(sb_eB, 0.0)

        # persistent intermediates (qhT/khT head-separated: all matmul
        # operands live at partition base 0 -- avoids PE row-group switching)
        sb_qhT = sing.tile([64, H, ROWS], BF16)
        sb_khT = sing.tile([64, H, HALO], BF16)
        sb_vh = sing.tile([128, 5, H, 65], BF16)  # shifted row tiles, 65 = 64d+1
        sb_concat = sing.tile([128, J, DM], BF16)
        sb_concatT = sing.tile([128, 4, ROWS], BF16)

        # ---------------- phase 1: projections ----------------
        with tc.tile_pool(name="ppj", bufs=2, space="PSUM") as ppj, \
             tc.tile_pool(name="ppv", bufs=2, space="PSUM") as ppv, \
             tc.tile_pool(name="ppt", bufs=1, space="PSUM") as ppt:
            # qhT[h][64 dout, rows]: head pairs per 2-bank psum tile
            for hp2 in range(4):
                ps = ppj.tile([64, 2, ROWS], F32, tag="pj")
                for i in range(2):
                    hh = 2 * hp2 + i
                    for kc in range(4):
                        nc.tensor.matmul(ps[:, i, :],
                                         sb_Wq[:, kc, 64 * hh:64 * hh + 64],
                                         sb_qT[:, kc, :], start=(kc == 0),
                                         stop=(kc == 3 and not with_bias))
                    if with_bias:
                        nc.tensor.matmul(ps[:, i, :],
                                         sb_bq[0:1, 64 * hh:64 * hh + 64],
                                         sb_ones_r, start=False, stop=True)
                nc.scalar.copy(sb_qhT[:, 2 * hp2:2 * hp2 + 2, :], ps)
            # khT[h][64 dout, halo]: main 512 cols paired; 8-col tails separate
            for hp2 in range(4):
                ps = ppj.tile([64, 2, ROWS], F32, tag="pj")
                pst = ppt.tile([64, 2, 8], F32, tag="pt")
                for i in range(2):
                    hh = 2 * hp2 + i
                    for kc in range(4):
                        nc.tensor.matmul(ps[:, i, :],
                                         sb_Wk[:, kc, 64 * hh:64 * hh + 64],
                                         sb_kT[:, kc, 0:512], start=(kc == 0),
                                         stop=(kc == 3 and not with_bias))
                        nc.tensor.matmul(pst[:, i, :],
                                         sb_Wk[:, kc, 64 * hh:64 * hh + 64],
                                         sb_kT[:, kc, 512:HALO], start=(kc == 0),
                                         stop=(kc == 3 and not with_bias))
                    if with_bias:
                        nc.tensor.matmul(ps[:, i, :],
                                         sb_bk[0:1, 64 * hh:64 * hh + 64],
                                         sb_ones_r, start=False, stop=True)
                        nc.tensor.matmul(pst[:, i, :],
                                         sb_bk[0:1, 64 * hh:64 * hh + 64],
                                         sb_ones_r[0:1, 0:8], start=False, stop=True)
                nc.vector.tensor_copy(sb_khT[:, 2 * hp2:2 * hp2 + 2, 0:512], ps)
                nc.vector.tensor_copy(sb_khT[:, 2 * hp2:2 * hp2 + 2, 512:HALO], pst)
            # vh[rows(+halo shift), dout] in 65-strided head blocks
            for rt in range(5):
                nr = 128 if rt < 4 else 8
                ps = ppv.tile([128, DM], F32, tag="pv")
                for kc in range(4):
                    nc.tensor.matmul(ps[0:nr, :],
                                     sb_vT[:, kc, 128 * rt:128 * rt + nr],
                                     sb_Wv[:, kc, :], start=(kc == 0),
                                     stop=(kc == 3 and not with_bias))
                if with_bias:
                    nc.tensor.matmul(ps[0:nr, :], sb_ones_r[0:1, 0:nr], sb_bv,
                                     start=False, stop=True)
                eng = nc.scalar.copy if rt % 2 == 0 else nc.vector.tensor_copy
                eng(sb_vh[0:nr, rt, :, 0:64],
                    ps[0:nr, :].rearrange("p (h d) -> p h d", h=H))
            nc.vector.memset(sb_vh[:, :, :, 64:65], 1.0)

        # ---------------- phase 2: band attention ----------------
        with tc.tile_pool(name="pSA", bufs=2, space="PSUM") as pSA, \
             tc.tile_pool(name="pSB", bufs=1, space="PSUM") as pSB, \
             tc.tile_pool(name="pN", bufs=2, space="PSUM") as pN, \
             tc.tile_pool(name="pW", bufs=1, space="PSUM") as pW, \
             tc.tile_pool(name="att", bufs=2) as att:
            # dense 512-row matmuls on scratch, issued where the PE queue
            # waits on exp/copies: keeps the HAM activity monitor above its
            # throttle threshold through the small-matmul attention phase
            pwt = pW.tile([128, 512], F32, tag="w")

            def ham_filler(n=2):
                for _ in range(n):
                    nc.tensor.matmul(pwt, sb_warm[:, 0:128], sb_warm,
                                     start=True, stop=True)

            for j in range(J if upto >= 2 else 0):
                psa = pSA.tile([128, H * 128], F32, tag="sa")
                psb = pSB.tile([8, H * 8], F32, tag="sb")
                for h in range(H):
                    qh_j = sb_qhT[:, h, 128 * j:128 * j + 128]
                    nc.tensor.matmul(psa[:, 128 * h:128 * h + 128],
                                     sb_khT[:, h, 128 * j:128 * j + 128],
                                     qh_j, start=True, stop=True)
                    # band of piece B only touches query cols 120:128
                    nc.tensor.matmul(psb[:, 8 * h:8 * h + 8],
                                     sb_khT[:, h, 128 * j + 128:128 * j + 136],
                                     sb_qhT[:, h, 128 * j + 120:128 * j + 128],
                                     start=True, stop=True)
                # CS = C * S (in place), E = exp(CS) - 1 -> SBUF (bf16)
                ca = sb_CA[:, j, :]
                ca_b = bass.AP(tensor=ca.tensor, offset=ca.offset,
                               ap=[list(ca.ap[0]), [0, H], list(ca.ap[1])])
                psa_v = psa[:].rearrange("p (h q) -> p h q", h=H)
                nc.vector.tensor_mul(psa_v, psa_v, ca_b)
                cb = sb_CBs[:, j, :]
                cb_b = bass.AP(tensor=cb.tensor, offset=cb.offset,
                               ap=[list(cb.ap[0]), [0, H], list(cb.ap[1])])
                psb_v = psb[:].rearrange("p (h q) -> p h q", h=H)
                nc.vector.tensor_mul(psb_v, psb_v, cb_b)
                ham_filler(2)
                eA = att.tile([128, H * 128], BF16, tag="eA")
                nc.scalar.activation(eA, psa, EXP)
                nc.vector.tensor_scalar_add(eA, eA, -1.0)
                eB_v = sb_eB[:].rearrange("p (h q) -> p h q", h=H)[:, :, 120:128]
                nc.scalar.activation(eB_v, psb_v, EXP)
                nc.scalar.activation(eB_v, eB_v,
                                     mybir.ActivationFunctionType.Copy, bias=-1.0)
                # numer: [128, 65] per head, 4 heads per psum tile
                pn0 = pN.tile([128, 4, 65], F32, tag="n")
                pn1 = pN.tile([128, 4, 65], F32, tag="n")
                for h in range(H):
                    pn = (pn0 if h < 4 else pn1)[:, h % 4, :]
                    nc.tensor.matmul(pn, eA[:, 128 * h:128 * h + 128],
                                     sb_vh[:, j, h, :], start=True, stop=False)
                    nc.tensor.matmul(pn, sb_eB[:, 128 * h:128 * h + 128],
                                     sb_vh[0:8, j + 1, h, :], start=False,
                                     stop=False)
                    nc.tensor.matmul(pn, sb_ones_r[0:1, 0:128],
                                     sb_vs65[0:1, h, :], start=False, stop=True)
                # divide + write concat
                r = att.tile([128, H], F32, tag="r")
                nc.vector.reciprocal(r[:, 0:4], pn0[:, :, 64])
                nc.vector.reciprocal(r[:, 4:8], pn1[:, :, 64])
                for pn, ho in ((pn0, 0), (pn1, 4)):
                    rs = r[:, ho:ho + 4]
                    r_b = bass.AP(tensor=rs.tensor, offset=rs.offset,
                                  ap=[list(rs.ap[0]), list(rs.ap[1]), [0, 64]])
                    outv = sb_concat[:, j, 64 * ho:64 * ho + 256]
                    nc.vector.scalar_tensor_tensor(
                        outv.rearrange("p (h d) -> p h d", h=4),
                        pn[:, :, 0:64], 1.0, r_b, op0=MULT, op1=MULT)

        # ---------------- phase 3: transpose concat ----------------
        # (PE transpose-mode does not register as busy for HAM, so keep a
        # real matmul in flight per dc group)
        with tc.tile_pool(name="pT", bufs=2, space="PSUM") as pT, \
             tc.tile_pool(name="pW3", bufs=1, space="PSUM") as pW3:
            pwt3 = pW3.tile([128, 512], F32, tag="w3")
            for dc in range(4 if upto >= 3 else 0):
                nc.tensor.matmul(pwt3, sb_warm[:, 0:128], sb_warm,
                                 start=True, stop=True)
                for rc in range(4):
                    pt = pT.tile([128, 128], BF16, tag="t")
                    nc.tensor.transpose(pt, sb_concat[:, rc, 128 * dc:128 * dc + 128],
                                        sb_id)
                    eng = nc.scalar.copy if (dc + rc) % 2 == 0 else nc.vector.tensor_copy
                    eng(sb_concatT[:, dc, 128 * rc:128 * rc + 128], pt)

        # ---------------- phase 4: output projection ----------------
        with tc.tile_pool(name="pF", bufs=2, space="PSUM") as pF, \
             tc.tile_pool(name="fout", bufs=2) as fout:
            if upto < 4:
                src_t = {2: sb_concat, 3: sb_concatT}.get(upto, sb_concatT)
                for rc in range(4):
                    if upto == 1:
                        nc.sync.dma_start(out_d[128 * rc:128 * rc + 128, 0:256],
                                          sb_vh[:, rc, 0:4, 0:64])
                    else:
                        nc.sync.dma_start(out_d[128 * rc:128 * rc + 128, :],
                                          src_t[:, rc, 0:DM])
            for rc in range(4 if upto >= 4 else 0):
                pf = pF.tile([128, DM], F32, tag="f")
                for dc in range(4):
                    nc.tensor.matmul(pf, sb_concatT[:, dc, 128 * rc:128 * rc + 128],
                                     sb_Wo[:, dc, :], start=(dc == 0), stop=(dc == 3))
                so = fout.tile([128, DM], BF16, tag="fo")
                nc.scalar.copy(so, pf)
                dq = nc.sync if rc % 2 == 0 else nc.scalar
                dq.dma_start(out_d[128 * rc:128 * rc + 128, :], so)

    if not nc.is_finalized():
        nc.finalize()
    return nc


_PROG_CACHE = {}


def _get_program(with_bias):
    import os
    upto = int(os.environ.get("BASS_KERNEL_UPTO", "4"))
    key = (bool(with_bias), upto)
    if key not in _PROG_CACHE:
        _PROG_CACHE[key] = _build_program(with_bias, upto)
    return _PROG_CACHE[key]


# ----------------------------------------------------------------------------
# entry point
# ----------------------------------------------------------------------------

def prep_in_maps(q, k, v, Wq, bq, Wk, bk, Wv, bv, Wo, bo, **_unused):
    """Builds per-core input maps + the traced program; returns (in_maps, nc)."""
    q = np.asarray(q, np.float32)
    k = np.asarray(k, np.float32)
    v = np.asarray(v, np.float32)
    Wq_b = np.ascontiguousarray(Wq, np.float32).astype(NP_BF16)
    Wk_b = np.ascontiguousarray(Wk, np.float32).astype(NP_BF16)
    Wv_b = np.ascontiguousarray(Wv, np.float32).astype(NP_BF16)
    Wo_b = np.ascontiguousarray(Wo, np.float32).astype(NP_BF16)
    bq = np.asarray(bq, np.float32).reshape(-1)
    bk = np.asarray(bk, np.float32).reshape(-1)
    bv = np.asarray(bv, np.float32).reshape(-1)
    bo = np.asarray(bo, np.float32).reshape(-1)
    with_bias = bool(np.any(bq) or np.any(bk) or np.any(bv))
    nc = _get_program(with_bias)

    C = _band_count() / np.float32(math.sqrt(DK))
    ident = np.eye(128, dtype=NP_BF16)
    # VsumW = (sum_k v[k]) @ Wv (+ seq*bv) in fp64, per batch
    vsum = v.sum(axis=1, dtype=np.float64)  # [2, 512]
    vsW = vsum @ Wv_b.astype(np.float64) + float(SEQ) * bv.astype(np.float64)

    in_maps = []
    for c in range(8):
        b, R0 = c // 4, ROWS * (c % 4)
        qT = np.ascontiguousarray(q[b, R0:R0 + ROWS, :].T).astype(NP_BF16)
        kT = np.zeros((DM, HALO), NP_BF16)
        vT = np.zeros((DM, HALO), NP_BF16)
        g0 = R0 - 4
        s0, s1 = max(g0, 0), min(R0 + ROWS + 4, SEQ)
        kT[:, s0 - g0:s1 - g0] = k[b, s0:s1, :].T.astype(NP_BF16)
        vT[:, s0 - g0:s1 - g0] = v[b, s0:s1, :].T.astype(NP_BF16)
        CA, CB = _c_tiles(R0, C)
        CBs = np.ascontiguousarray(CB[:, :, 120:128]).astype(NP_BF16)
        assert not CB[:, :, :120].any()
        vs65 = np.empty((H, 65), np.float64)
        vs65[:, 0:64] = vsW[b].reshape(H, 64)
        vs65[:, 64] = float(SEQ)
        m = {"qT": qT, "kT": kT, "vT": vT, "Wq": Wq_b, "Wk": Wk_b, "Wv": Wv_b,
             "Wo": Wo_b, "CA": CA.astype(NP_BF16), "CBs": CBs,
             "vsW65": vs65.reshape(1, H * 65).astype(NP_BF16),
             "ident": ident}
        if with_bias:
            m["bq"] = bq[None, :].astype(NP_BF16)
            m["bk"] = bk[None, :].astype(NP_BF16)
            m["bv"] = bv[None, :].astype(NP_BF16)
        in_maps.append(m)
    return in_maps, nc


def kernel(q, k, v, Wq, bq, Wk, bk, Wv, bv, Wo, bo, **_unused):
    bo = np.asarray(bo, np.float32).reshape(-1)
    in_maps, nc = prep_in_maps(q, k, v, Wq, bq, Wk, bk, Wv, bv, Wo, bo)
    res = run_bass_kernel_spmd(nc, in_maps, core_ids=list(range(8)))
    out = np.empty((2, SEQ, DM), np.float32)
    for c in range(8):
        b, R0 = c // 4, ROWS * (c % 4)
        out[b, R0:R0 + ROWS, :] = res.results[c]["out"].astype(np.float32)
    if np.any(bo):
        out += bo
    return out


if __name__ == "__main__":
    rng = np.random.default_rng(0)
    s = 1.0 / math.sqrt(DM)
    inp = dict(
        q=rng.standard_normal((2, SEQ, DM)).astype(np.float32),
        k=rng.standard_normal((2, SEQ, DM)).astype(np.float32),
        v=rng.standard_normal((2, SEQ, DM)).astype(np.float32),
        Wq=(rng.standard_normal((DM, DM)) * s).astype(np.float32),
        bq=np.zeros(DM, np.float32),
        Wk=(rng.standard_normal((DM, DM)) * s).astype(np.float32),
        bk=np.zeros(DM, np.float32),
        Wv=(rng.standard_normal((DM, DM)) * s).astype(np.float32),
        bv=np.zeros(DM, np.float32),
        Wo=(rng.standard_normal((DM, DM)) * s).astype(np.float32),
        bo=np.zeros(DM, np.float32),
    )
    out = kernel(**inp)
    print("kernel ran, out shape", out.shape, "mean", np.abs(out).mean())


# revision 67
# speedup vs baseline: 1.0343x; 1.0133x over previous
"""Trainium2 Bass kernel for nn_MultiHeadLocalAttention (band-limited attention).

Math: scores are multiplied by a band-count matrix C that is zero outside
|q-k|<=4, then a FULL-row softmax is applied.  Out-of-band entries contribute
exp(0)=1, so with E = exp(C*S) over the 136-wide halo band:

  out[q]   = (sum_halo (E[k,q]-1) vh[k] + VsumW) / denom[q]
  denom[q] = sum_halo (E[k,q]-1) + 2048

where VsumW = (sum_k v[k]) @ Wv (computed host-side in fp64).  This makes the
O(seq^2) attention an O(seq*band) computation, exact to fp rounding.

Sharding: batch*seq rows split across 8 cores (512 rows each); each core
computes all 8 heads for its rows.  k/v inputs carry a +-4 halo.  All matmul
operands are bf16 (1 PE cycle/row vs 4 for fp32); PSUM accumulation is fp32.
"""

import math
import sys
from contextlib import ExitStack

import numpy as np

sys.path.insert(0, "/opt/trn_rl_repo")

import ml_dtypes

import concourse.bass as bass
import concourse.tile as tile
from concourse import bacc, mybir
from concourse.bass_utils import run_bass_kernel_spmd

F32 = mybir.dt.float32
BF16 = mybir.dt.bfloat16
NP_BF16 = ml_dtypes.bfloat16
SEQ, DM, H, DK = 2048, 512, 8, 64
ROWS = 512          # query rows per core
HALO = ROWS + 8     # padded k/v halo columns per core
J = 4               # 128-row query tiles per core
WBAND = 2


# ----------------------------------------------------------------------------
# host-side helpers
# ----------------------------------------------------------------------------

def _band_count(seq=SEQ, window=WBAND):
    i = np.arange(seq)
    lo = np.clip(i - window, 0, None)
    hi = np.clip(i + window, None, seq - 1)
    lo = np.where(i == 1, 0, lo)
    hi = np.where(i == 1, window + 1, hi)
    lo = np.where(i == seq - 2, seq - window - 2, lo)
    hi = np.where(i == seq - 2, seq - 1, hi)
    a = np.arange(seq)[None, :]
    M = ((a >= lo[:, None]) & (a <= hi[:, None])).astype(np.float32)
    return M.T @ M


def _c_tiles(R0, C):
    """CA [J,128,128], CB [J,8,128] with C[k,q]/sqrt(dk) (C symmetric)."""
    CA = np.zeros((J, 128, 128), np.float32)
    CB = np.zeros((J, 8, 128), np.float32)
    for j in range(J):
        qg = R0 + 128 * j + np.arange(128)
        kgA = R0 - 4 + 128 * j + np.arange(128)
        kgB = R0 - 4 + 128 * j + 128 + np.arange(8)
        mA = (kgA >= 0) & (kgA < SEQ)
        mB = (kgB >= 0) & (kgB < SEQ)
        CA[j][mA, :] = C[np.ix_(kgA[mA], qg)]
        CB[j][mB, :] = C[np.ix_(kgB[mB], qg)]
    return CA, CB


# ----------------------------------------------------------------------------
# device program
# ----------------------------------------------------------------------------

def _build_program(with_bias, upto=4):
    nc = bacc.Bacc()
    E_IN = dict(kind="ExternalInput")
    qT_d = nc.dram_tensor("qT", [DM, ROWS], BF16, **E_IN)
    kT_d = nc.dram_tensor("kT", [DM, HALO], BF16, **E_IN)
    vT_d = nc.dram_tensor("vT", [DM, HALO], BF16, **E_IN)
    Wq_d = nc.dram_tensor("Wq", [DM, DM], BF16, **E_IN)
    Wk_d = nc.dram_tensor("Wk", [DM, DM], BF16, **E_IN)
    Wv_d = nc.dram_tensor("Wv", [DM, DM], BF16, **E_IN)
    Wo_d = nc.dram_tensor("Wo", [DM, DM], BF16, **E_IN)
    CA_d = nc.dram_tensor("CA", [J, 128, 128], BF16, **E_IN)
    CBs_d = nc.dram_tensor("CBs", [J, 8, 8], BF16, **E_IN)
    vsW_d = nc.dram_tensor("vsW65", [1, H * 65], BF16, **E_IN)
    ident_d = nc.dram_tensor("ident", [128, 128], BF16, **E_IN)
    if with_bias:
        bq_d = nc.dram_tensor("bq", [1, DM], BF16, **E_IN)
        bk_d = nc.dram_tensor("bk", [1, DM], BF16, **E_IN)
        bv_d = nc.dram_tensor("bv", [1, DM], BF16, **E_IN)
    out_d = nc.dram_tensor("out", [ROWS, DM], BF16, kind="ExternalOutput")

    MULT = mybir.AluOpType.mult
    EXP = mybir.ActivationFunctionType.Exp

    with tile.TileContext(nc) as tc, ExitStack() as ctx:
        sing = ctx.enter_context(tc.tile_pool(name="sing", bufs=1))

        # --- load everything to SBUF (q-proj operands first), split across
        # the sync and scalar HWDGE queues; small tables ride gpsimd SWDGE ---
        sb_Wq = sing.tile([128, 4, DM], BF16)
        nc.sync.dma_start(sb_Wq, Wq_d[:].rearrange("(kc p) n -> p kc n", p=128))
        sb_qT = sing.tile([128, 4, ROWS], BF16)
        nc.scalar.dma_start(sb_qT, qT_d[:].rearrange("(kc p) r -> p kc r", p=128))
        sb_kT = sing.tile([128, 4, HALO], BF16)
        nc.sync.dma_start(sb_kT, kT_d[:].rearrange("(kc p) r -> p kc r", p=128))
        sb_Wk = sing.tile([128, 4, DM], BF16)
        nc.scalar.dma_start(sb_Wk, Wk_d[:].rearrange("(kc p) n -> p kc n", p=128))
        sb_Wv = sing.tile([128, 4, DM], BF16)
        nc.sync.dma_start(sb_Wv, Wv_d[:].rearrange("(kc p) n -> p kc n", p=128))
        sb_vT = sing.tile([128, 4, HALO], BF16)
        nc.scalar.dma_start(sb_vT, vT_d[:].rearrange("(kc p) r -> p kc r", p=128))
        sb_CA = sing.tile([128, J, 128], BF16)
        nc.sync.dma_start(sb_CA, CA_d[:].rearrange("j p q -> p j q"))
        sb_Wo = sing.tile([128, 4, DM], BF16)
        nc.scalar.dma_start(sb_Wo, Wo_d[:].rearrange("(kc p) n -> p kc n", p=128))
        sb_CBs = sing.tile([8, J, 8], BF16)
        nc.gpsimd.dma_start(sb_CBs, CBs_d[:].rearrange("j p q -> p j q"))
        sb_vs65 = sing.tile([1, H, 65], BF16)     # [VsumW_h, 2048] per head
        nc.gpsimd.dma_start(sb_vs65, vsW_d[:].rearrange("p (h d) -> p h d", h=H))
        sb_id = sing.tile([128, 128], BF16)
        nc.gpsimd.dma_start(sb_id, ident_d[:])
        if with_bias:
            sb_bq = sing.tile([1, DM], BF16)
            nc.gpsimd.dma_start(sb_bq, bq_d[:])
            sb_bk = sing.tile([1, DM], BF16)
            nc.gpsimd.dma_start(sb_bk, bk_d[:])
            sb_bv = sing.tile([1, DM], BF16)
            nc.gpsimd.dma_start(sb_bv, bv_d[:])

        sb_ones_r = sing.tile([1, ROWS], BF16)    # ones row (rhs for bias/Vsum)
        nc.vector.memset(sb_ones_r, 1.0)
        sb_eB = sing.tile([8, H * 128], BF16)     # exp(C*S)-1 piece B; 0 off band
        nc.vector.memset(sb_eB, 0.0)

        # persistent intermediates (qhT/khT head-separated: all matmul
        # operands live at partition base 0 -- avoids PE row-group switching)
        sb_qhT = sing.tile([64, H, ROWS], BF16)
        sb_khT = sing.tile([64, H, HALO], BF16)
        sb_vh = sing.tile([128, 5, H, 65], BF16)  # shifted row tiles, 65 = 64d+1
        sb_concat = sing.tile([128, J, DM], BF16)
        sb_concatT = sing.tile([128, 4, ROWS], BF16)

        # ---------------- phase 1: projections ----------------
        with tc.tile_pool(name="ppj", bufs=2, space="PSUM") as ppj, \
             tc.tile_pool(name="ppv", bufs=2, space="PSUM") as ppv, \
             tc.tile_pool(name="ppt", bufs=1, space="PSUM") as ppt:
            # qhT[h][64 dout, rows]: head pairs per 2-bank psum tile
            for hp2 in range(4):
                ps = ppj.tile([64, 2, ROWS], F32, tag="pj")
                for i in range(2):
                    hh = 2 * hp2 + i
                    for kc in range(4):
                        nc.tensor.matmul(ps[:, i, :],
                                         sb_Wq[:, kc, 64 * hh:64 * hh + 64],
                                         sb_qT[:, kc, :], start=(kc == 0),
                                         stop=(kc == 3 and not with_bias))
                    if with_bias:
                        nc.tensor.matmul(ps[:, i, :],
                                         sb_bq[0:1, 64 * hh:64 * hh + 64],
                                         sb_ones_r, start=False, stop=True)
                nc.scalar.copy(sb_qhT[:, 2 * hp2:2 * hp2 + 2, :], ps)
            # khT[h][64 dout, halo]: main 512 cols paired; 8-col tails separate
            for hp2 in range(4):
                ps = ppj.tile([64, 2, ROWS], F32, tag="pj")
                pst = ppt.tile([64, 2, 8], F32, tag="pt")
                for i in range(2):
                    hh = 2 * hp2 + i
                    for kc in range(4):
                        nc.tensor.matmul(ps[:, i, :],
                                         sb_Wk[:, kc, 64 * hh:64 * hh + 64],
                                         sb_kT[:, kc, 0:512], start=(kc == 0),
                                         stop=(kc == 3 and not with_bias))
                        nc.tensor.matmul(pst[:, i, :],
                                         sb_Wk[:, kc, 64 * hh:64 * hh + 64],
                                         sb_kT[:, kc, 512:HALO], start=(kc == 0),
                                         stop=(kc == 3 and not with_bias))
                    if with_bias:
                        nc.tensor.matmul(ps[:, i, :],
                                         sb_bk[0:1, 64 * hh:64 * hh + 64],
                                         sb_ones_r, start=False, stop=True)
                        nc.tensor.matmul(pst[:, i, :],
                                         sb_bk[0:1, 64 * hh:64 * hh + 64],
                                         sb_ones_r[0:1, 0:8], start=False, stop=True)
                nc.vector.tensor_copy(sb_khT[:, 2 * hp2:2 * hp2 + 2, 0:512], ps)
                nc.vector.tensor_copy(sb_khT[:, 2 * hp2:2 * hp2 + 2, 512:HALO], pst)
            # vh[rows(+halo shift), dout] in 65-strided head blocks
            for rt in range(5):
                nr = 128 if rt < 4 else 8
                ps = ppv.tile([128, DM], F32, tag="pv")
                for kc in range(4):
                    nc.tensor.matmul(ps[0:nr, :],
                                     sb_vT[:, kc, 128 * rt:128 * rt + nr],
                                     sb_Wv[:, kc, :], start=(kc == 0),
                                     stop=(kc == 3 and not with_bias))
                if with_bias:
                    nc.tensor.matmul(ps[0:nr, :], sb_ones_r[0:1, 0:nr], sb_bv,
                                     start=False, stop=True)
                eng = nc.scalar.copy if rt % 2 == 0 else nc.vector.tensor_copy
                eng(sb_vh[0:nr, rt, :, 0:64],
                    ps[0:nr, :].rearrange("p (h d) -> p h d", h=H))
            nc.vector.memset(sb_vh[:, :, :, 64:65], 1.0)

        # ---------------- phase 2: band attention ----------------
        with tc.tile_pool(name="pSA", bufs=2, space="PSUM") as pSA, \
             tc.tile_pool(name="pSB", bufs=1, space="PSUM") as pSB, \
             tc.tile_pool(name="pN", bufs=3, space="PSUM") as pN, \
             tc.tile_pool(name="att", bufs=2) as att:
            for j in range(J if upto >= 2 else 0):
                psa = pSA.tile([128, H * 128], F32, tag="sa")
                psb = pSB.tile([8, H * 8], F32, tag="sb")
                for h in range(H):
                    qh_j = sb_qhT[:, h, 128 * j:128 * j + 128]
                    nc.tensor.matmul(psa[:, 128 * h:128 * h + 128],
                                     sb_khT[:, h, 128 * j:128 * j + 128],
                                     qh_j, start=True, stop=True)
                    # band of piece B only touches query cols 120:128
                    nc.tensor.matmul(psb[:, 8 * h:8 * h + 8],
                                     sb_khT[:, h, 128 * j + 128:128 * j + 136],
                                     sb_qhT[:, h, 128 * j + 120:128 * j + 128],
                                     start=True, stop=True)
                # CS = C * S (in place), E = exp(CS) - 1 -> SBUF (bf16)
                ca = sb_CA[:, j, :]
                ca_b = bass.AP(tensor=ca.tensor, offset=ca.offset,
                               ap=[list(ca.ap[0]), [0, H], list(ca.ap[1])])
                psa_v = psa[:].rearrange("p (h q) -> p h q", h=H)
                nc.vector.tensor_mul(psa_v, psa_v, ca_b)
                cb = sb_CBs[:, j, :]
                cb_b = bass.AP(tensor=cb.tensor, offset=cb.offset,
                               ap=[list(cb.ap[0]), [0, H], list(cb.ap[1])])
                psb_v = psb[:].rearrange("p (h q) -> p h q", h=H)
                nc.vector.tensor_mul(psb_v, psb_v, cb_b)
                eA = att.tile([128, H * 128], BF16, tag="eA")
                nc.scalar.activation(eA, psa, EXP)
                nc.vector.tensor_scalar_add(eA, eA, -1.0)
                eB_v = sb_eB[:].rearrange("p (h q) -> p h q", h=H)[:, :, 120:128]
                nc.scalar.activation(eB_v, psb_v, EXP)
                nc.scalar.activation(eB_v, eB_v,
                                     mybir.ActivationFunctionType.Copy, bias=-1.0)
                # numer: [128, 65] per head, 4 heads per psum tile
                pn0 = pN.tile([128, 4, 65], F32, tag="n")
                pn1 = pN.tile([128, 4, 65], F32, tag="n")
                for h in range(H):
                    pn = (pn0 if h < 4 else pn1)[:, h % 4, :]
                    nc.tensor.matmul(pn, eA[:, 128 * h:128 * h + 128],
                                     sb_vh[:, j, h, :], start=True, stop=False)
                    nc.tensor.matmul(pn, sb_eB[:, 128 * h:128 * h + 128],
                                     sb_vh[0:8, j + 1, h, :], start=False,
                                     stop=False)
                    nc.tensor.matmul(pn, sb_ones_r[0:1, 0:128],
                                     sb_vs65[0:1, h, :], start=False, stop=True)
                # divide + write concat
                r = att.tile([128, H], F32, tag="r")
                nc.vector.reciprocal(r[:, 0:4], pn0[:, :, 64])
                nc.vector.reciprocal(r[:, 4:8], pn1[:, :, 64])
                for pn, ho in ((pn0, 0), (pn1, 4)):
                    rs = r[:, ho:ho + 4]
                    r_b = bass.AP(tensor=rs.tensor, offset=rs.offset,
                                  ap=[list(rs.ap[0]), list(rs.ap[1]), [0, 64]])
                    outv = sb_concat[:, j, 64 * ho:64 * ho + 256]
                    nc.vector.scalar_tensor_tensor(
                        outv.rearrange("p (h d) -> p h d", h=4),
                        pn[:, :, 0:64], 1.0, r_b, op0=MULT, op1=MULT)

        # ---------------- phase 3: transpose concat ----------------
        with tc.tile_pool(name="pT", bufs=2, space="PSUM") as pT:
            for dc in range(4 if upto >= 3 else 0):
                for rc in range(4):
                    pt = pT.tile([128, 128], BF16, tag="t")
                    nc.tensor.transpose(pt, sb_concat[:, rc, 128 * dc:128 * dc + 128],
                                        sb_id)
                    eng = nc.scalar.copy if (dc + rc) % 2 == 0 else nc.vector.tensor_copy
                    eng(sb_concatT[:, dc, 128 * rc:128 * rc + 128], pt)

        # ---------------- phase 4: output projection ----------------
        with tc.tile_pool(name="pF", bufs=2, space="PSUM") as pF, \
             tc.tile_pool(name="fout", bufs=2) as fout:
            if upto < 4:
                src_t = {2: sb_concat, 3: sb_concatT}.get(upto, sb_concatT)
                for rc in range(4):
                    if upto == 1:
                        nc.sync.dma_start(out_d[128 * rc:128 * rc + 128, 0:256],
                                          sb_vh[:, rc, 0:4, 0:64])
                    else:
                        nc.sync.dma_start(out_d[128 * rc:128 * rc + 128, :],
                                          src_t[:, rc, 0:DM])
            for rc in range(4 if upto >= 4 else 0):
                pf = pF.tile([128, DM], F32, tag="f")
                for dc in range(4):
                    nc.tensor.matmul(pf, sb_concatT[:, dc, 128 * rc:128 * rc + 128],
                                     sb_Wo[:, dc, :], start=(dc == 0), stop=(dc == 3))
                so = fout.tile([128, DM], BF16, tag="fo")
                nc.scalar.copy(so, pf)
                dq = nc.sync if rc % 2 == 0 else nc.scalar
                dq.dma_start(out_d[128 * rc:128 * rc + 128, :], so)

    if not nc.is_finalized():
        nc.finalize()
    return nc


_PROG_CACHE = {}


def _get_program(with_bias):
    import os
    upto = int(os.environ.get("BASS_KERNEL_UPTO", "4"))
    key = (bool(with_bias), upto)
    if key not in _PROG_CACHE:
        _PROG_CACHE[key] = _build_program(with_bias, upto)
    return _PROG_CACHE[key]


# ----------------------------------------------------------------------------
# entry point
# ----------------------------------------------------------------------------

def prep_in_maps(q, k, v, Wq, bq, Wk, bk, Wv, bv, Wo, bo, **_unused):
    """Builds per-core input maps + the traced program; returns (in_maps, nc)."""
    q = np.asarray(q, np.float32)
    k = np.asarray(k, np.float32)
    v = np.asarray(v, np.float32)
    Wq_b = np.ascontiguousarray(Wq, np.float32).astype(NP_BF16)
    Wk_b = np.ascontiguousarray(Wk, np.float32).astype(NP_BF16)
    Wv_b = np.ascontiguousarray(Wv, np.float32).astype(NP_BF16)
    Wo_b = np.ascontiguousarray(Wo, np.float32).astype(NP_BF16)
    bq = np.asarray(bq, np.float32).reshape(-1)
    bk = np.asarray(bk, np.float32).reshape(-1)
    bv = np.asarray(bv, np.float32).reshape(-1)
    bo = np.asarray(bo, np.float32).reshape(-1)
    with_bias = bool(np.any(bq) or np.any(bk) or np.any(bv))
    nc = _get_program(with_bias)

    C = _band_count() / np.float32(math.sqrt(DK))
    ident = np.eye(128, dtype=NP_BF16)
    # VsumW = (sum_k v[k]) @ Wv (+ seq*bv) in fp64, per batch
    vsum = v.sum(axis=1, dtype=np.float64)  # [2, 512]
    vsW = vsum @ Wv_b.astype(np.float64) + float(SEQ) * bv.astype(np.float64)

    in_maps = []
    for c in range(8):
        b, R0 = c // 4, ROWS * (c % 4)
        qT = np.ascontiguousarray(q[b, R0:R0 + ROWS, :].T).astype(NP_BF16)
        kT = np.zeros((DM, HALO), NP_BF16)
        vT = np.zeros((DM, HALO), NP_BF16)
        g0 = R0 - 4
        s0, s1 = max(g0, 0), min(R0 + ROWS + 4, SEQ)
        kT[:, s0 - g0:s1 - g0] = k[b, s0:s1, :].T.astype(NP_BF16)
        vT[:, s0 - g0:s1 - g0] = v[b, s0:s1, :].T.astype(NP_BF16)
        CA, CB = _c_tiles(R0, C)
        CBs = np.ascontiguousarray(CB[:, :, 120:128]).astype(NP_BF16)
        assert not CB[:, :, :120].any()
        vs65 = np.empty((H, 65), np.float64)
        vs65[:, 0:64] = vsW[b].reshape(H, 64)
        vs65[:, 64] = float(SEQ)
        m = {"qT": qT, "kT": kT, "vT": vT, "Wq": Wq_b, "Wk": Wk_b, "Wv": Wv_b,
             "Wo": Wo_b, "CA": CA.astype(NP_BF16), "CBs": CBs,
             "vsW65": vs65.reshape(1, H * 65).astype(NP_BF16),
             "ident": ident}
        if with_bias:
            m["bq"] = bq[None, :].astype(NP_BF16)
            m["bk"] = bk[None, :].astype(NP_BF16)
            m["bv"] = bv[None, :].astype(NP_BF16)
        in_maps.append(m)
    return in_maps, nc


def kernel(q, k, v, Wq, bq, Wk, bk, Wv, bv, Wo, bo, **_unused):
    bo = np.asarray(bo, np.float32).reshape(-1)
    in_maps, nc = prep_in_maps(q, k, v, Wq, bq, Wk, bk, Wv, bv, Wo, bo)
    res = run_bass_kernel_spmd(nc, in_maps, core_ids=list(range(8)))
    out = np.empty((2, SEQ, DM), np.float32)
    for c in range(8):
        b, R0 = c // 4, ROWS * (c % 4)
        out[b, R0:R0 + ROWS, :] = res.results[c]["out"].astype(np.float32)
    if np.any(bo):
        out += bo
    return out


if __name__ == "__main__":
    rng = np.random.default_rng(0)
    s = 1.0 / math.sqrt(DM)
    inp = dict(
        q=rng.standard_normal((2, SEQ, DM)).astype(np.float32),
        k=rng.standard_normal((2, SEQ, DM)).astype(np.float32),
        v=rng.standard_normal((2, SEQ, DM)).astype(np.float32),
        Wq=(rng.standard_normal((DM, DM)) * s).astype(np.float32),
        bq=np.zeros(DM, np.float32),
        Wk=(rng.standard_normal((DM, DM)) * s).astype(np.float32),
        bk=np.zeros(DM, np.float32),
        Wv=(rng.standard_normal((DM, DM)) * s).astype(np.float32),
        bv=np.zeros(DM, np.float32),
        Wo=(rng.standard_normal((DM, DM)) * s).astype(np.float32),
        bo=np.zeros(DM, np.float32),
    )
    out = kernel(**inp)
    print("kernel ran, out shape", out.shape, "mean", np.abs(out).mean())
